# revision 4
# baseline (speedup 1.0000x reference)
"""TRN2 Bass kernel for nn_DiffusionUNet_64 (moe_routing).

Computation per sample b:
    pooled = mean(x[b], HW)                       (CIN,)
    rw = softmax(router(pooled, time_emb[b]))     (E,)
    w_eff = sum_e rw[e] * weight[e]               (COUT, CIN, 3, 3)
    y[b] = conv2d(x[b], w_eff, pad=1)             (COUT, H, W)

Sharding: data-parallel over batch, 4 samples per core on 8 cores.
The conv runs as 9 shifted fp16 matmuls (fp32 PSUM accumulation),
sample-major.  Sample 0 walks offsets outer-loop so the PE consumes
weight-DMA chunks as they arrive; samples 1-3 run m-sequential so PSUM
drains overlap the remaining matmul stream.  Expert mixing uses the
delta identity (softmax weights sum to 1): weff = W0 + sum_e s_e*(We-W0)
as 3 fused scalar-MAD ops on DVE (fp16, 4x mode).  The router runs in
fp32 on-device; pooling is split across DVE/ACT/Pool per sample.
Dummy warm-up matmuls keep the PE p-state ramped before the conv.
"""
import numpy as np

import concourse.bass as bass
import concourse.tile as tile
from concourse import bacc, mybir
from concourse.bass_utils import run_bass_kernel_spmd

F32 = mybir.dt.float32
FP16 = mybir.dt.float16

B, CIN, COUT, H, W = 32, 256, 256, 32, 32
E, TDIM, HID = 4, 256, 64
NCORES = 8
BLOC = B // NCORES          # 4 samples per core
NCH = CIN // 128            # 2 cin chunks
MCH = COUT // 128           # 2 cout chunks
HP, WP = H + 2, W + 2       # 34x34 padded
PIX = H * W                 # 1024
NPARAM = 528


def build_program():
    nc = bacc.Bacc("TRN2", target_bir_lowering=False, debug=False,
                   num_devices=NCORES)
    xp_d = nc.dram_tensor("xpad", [BLOC, 128, NCH, HP * WP], FP16,
                          kind="ExternalInput").ap()
    te_d = nc.dram_tensor("temb", [128, NCH, BLOC], F32, kind="ExternalInput").ap()
    wt_d = nc.dram_tensor("wt", [128, 9, NCH, E, COUT], FP16,
                          kind="ExternalInput").ap()
    rp_d = nc.dram_tensor("rparams", [128, NPARAM], F32, kind="ExternalInput").ap()
    out_d = nc.dram_tensor("out", [BLOC, MCH, 128, PIX], FP16,
                           kind="ExternalOutput").ap()

    AF = mybir.ActivationFunctionType
    ALU = mybir.AluOpType

    with tile.TileContext(nc) as tc:
        with tc.tile_pool(name="persist", bufs=1) as pp, \
             tc.tile_pool(name="weff", bufs=2) as wp, \
             tc.tile_pool(name="rwork", bufs=4) as rwk, \
             tc.tile_pool(name="osb", bufs=4) as ob, \
             tc.tile_pool(name="ps", bufs=7, space="PSUM") as ps, \
             tc.tile_pool(name="psd", bufs=1, space="PSUM") as psd:

            # ---- persistent tiles + input DMAs (x0 first: router-0 is the
            # critical path; weights follow; x1-3 interleave)
            xp = pp.tile([128, BLOC, NCH, HP * WP], FP16)
            te = pp.tile([128, NCH, BLOC], F32)
            rp = pp.tile([128, NPARAM], F32)
            wt = pp.tile([128, 9, NCH, E, COUT], FP16)

            nc.sync.dma_start(xp[:, 0, 0], xp_d[0, :, 0])
            nc.sync.dma_start(xp[:, 0, 1], xp_d[0, :, 1])
            nc.sync.dma_start(te[:], te_d[:])
            nc.sync.dma_start(rp[:], rp_d[:])
            nc.sync.dma_start(wt[:, 0:1], wt_d[:, 0:1])
            nc.sync.dma_start(wt[:, 1:2], wt_d[:, 1:2])
            nc.sync.dma_start(wt[:, 2:3], wt_d[:, 2:3])
            nc.sync.dma_start(xp[:, 1], xp_d[1])
            nc.sync.dma_start(wt[:, 3:4], wt_d[:, 3:4])
            nc.sync.dma_start(wt[:, 4:5], wt_d[:, 4:5])
            nc.sync.dma_start(xp[:, 2], xp_d[2])
            nc.sync.dma_start(wt[:, 5:6], wt_d[:, 5:6])
            nc.sync.dma_start(wt[:, 6:7], wt_d[:, 6:7])
            nc.sync.dma_start(xp[:, 3], xp_d[3])
            nc.sync.dma_start(wt[:, 7:8], wt_d[:, 7:8])
            nc.sync.dma_start(wt[:, 8:9], wt_d[:, 8:9])

            # ---- small constants
            ones1 = pp.tile([1, 128], F32)
            nc.vector.memset(ones1[:], 1.0)
            wdum = pp.tile([128, 512], FP16)
            nc.vector.memset(wdum[:], 0.001)
            xm_pre = []
            for b in range(BLOC):
                xmt = pp.tile([HID + 1, 1], F32, name=f"xm_{b}")
                nc.vector.memset(xmt[HID:HID + 1, :], 1.0)
                xm_pre.append(xmt)
            pscr = pp.tile([128, PIX], F32)

            # ---- PE warm-up dummies keep the p-state ramp alive
            psdum = psd.tile([128, 512], F32, tag="psd")

            def dummies(n):
                for _ in range(n):
                    nc.tensor.matmul(psdum[:], wdum[:, 0:128], wdum[:],
                                     start=True, stop=True)

            def center(b, c):
                # [128, 32, 32] view of the unpadded pixels of chunk c
                return xp[:, b, c].rearrange("p (h w) -> p h w", h=HP)[
                    :, 1:H + 1, 1:W + 1]

            pooled = [pp.tile([128, NCH], F32, name=f"pooled_{b}")
                      for b in range(BLOC)]
            rwbs = [None] * BLOC

            def emit_pooled(b, eng):
                for c in range(NCH):
                    if eng == "dve":
                        nc.vector.tensor_reduce(pooled[b][:, c:c + 1],
                                                center(b, c),
                                                mybir.AxisListType.XY, ALU.add)
                    elif eng == "act":
                        nc.scalar.activation(pscr[:], center(b, c),
                                             AF.Identity,
                                             accum_out=pooled[b][:, c:c + 1])
                    else:
                        nc.gpsimd.tensor_reduce(pooled[b][:, c:c + 1],
                                                center(b, c),
                                                mybir.AxisListType.XY, ALU.add)

            def emit_router(b):
                """Router math for sample b; rwb ready at the end."""
                rq = ps.tile([HID, 1], F32, tag="ps8", name=f"rq_{b}")
                for c in range(NCH):
                    nc.tensor.matmul(rq[:], rp[:, c * HID:(c + 1) * HID],
                                     te[:, c, b:b + 1],
                                     start=(c == 0), stop=(c == NCH - 1))
                rk = ps.tile([HID, 1], F32, tag="ps8", name=f"rk_{b}")
                for c in range(NCH):
                    nc.tensor.matmul(rk[:], rp[:, 128 + c * HID:128 + (c + 1) * HID],
                                     pooled[b][:, c:c + 1],
                                     start=(c == 0), stop=(c == NCH - 1))
                rv = ps.tile([HID, 1], F32, tag="ps8", name=f"rv_{b}")
                for c in range(NCH):
                    nc.tensor.matmul(rv[:], rp[:, 256 + c * HID:256 + (c + 1) * HID],
                                     pooled[b][:, c:c + 1],
                                     start=(c == 0), stop=(c == NCH - 1))
                q = rwk.tile([HID, 1], F32, tag="qs", name=f"qs_{b}")
                nc.vector.tensor_scalar_add(q[:], rq[:], rp[0:HID, 516:517])
                t1 = rwk.tile([HID, 1], F32, tag="t1", name=f"t1_{b}")
                nc.vector.scalar_tensor_tensor(t1[:], rk[:], rp[0:HID, 517:518],
                                               q[:], ALU.add, ALU.mult)
                attn = rwk.tile([HID, 1], F32, tag="attn", name=f"attn_{b}")
                nc.scalar.activation(attn[:], t1[:], AF.Sigmoid)
                xa = rwk.tile([HID, 1], F32, tag="xa", name=f"xa_{b}")
                nc.vector.scalar_tensor_tensor(xa[:], rv[:], rp[0:HID, 518:519],
                                               attn[:], ALU.add, ALU.mult)
                rh1 = ps.tile([HID, 1], F32, tag="ps8", name=f"rh1_{b}")
                nc.tensor.matmul(rh1[:], rp[0:HID, 384:448], xa[:],
                                 start=True, stop=True)
                h1s = rwk.tile([HID, 1], F32, tag="h1s", name=f"h1s_{b}")
                nc.scalar.activation(h1s[:], rh1[:], AF.Silu,
                                     bias=rp[0:HID, 519:520])
                rh2 = ps.tile([HID, 1], F32, tag="ps8", name=f"rh2_{b}")
                nc.tensor.matmul(rh2[:], rp[0:HID, 448:512], h1s[:],
                                 start=True, stop=True)
                xm = xm_pre[b]
                nc.vector.scalar_tensor_tensor(xm[0:HID, :], rh2[:],
                                               rp[0:HID, 520:521], xa[:],
                                               ALU.add, ALU.add)
                rl = ps.tile([1, E], F32, tag="ps8", name=f"rl_{b}")
                nc.tensor.matmul(rl[:], xm[:], rp[0:HID + 1, 512:516],
                                 start=True, stop=True)
                exps = rwk.tile([1, E], F32, tag="exps", name=f"exps_{b}")
                nc.scalar.activation(exps[:], rl[:], AF.Exp)
                rwp = ps.tile([128, E], F32, tag="ps8", name=f"rwp_{b}")
                nc.tensor.matmul(rwp[:], ones1[:], exps[:],
                                 start=True, stop=True)
                ssum = rwk.tile([128, 1], F32, tag="ssum", name=f"ssum_{b}")
                nc.vector.tensor_reduce(ssum[:], rwp[:], mybir.AxisListType.X,
                                        ALU.add)
                srec = rwk.tile([128, 1], F32, tag="srec", name=f"srec_{b}")
                nc.vector.reciprocal(srec[:], ssum[:])
                rwb = pp.tile([128, E], F32, name=f"rwb_{b}")
                nc.vector.tensor_scalar_mul(rwb[:], rwp[:], srec[:])
                rwbs[b] = rwb

            def mix_weff(b, o):
                rwb = rwbs[b]
                acc1 = wp.tile([128, NCH, COUT], FP16, tag="macc1",
                               name=f"acc1_{b}_{o}")
                nc.vector.scalar_tensor_tensor(acc1[:], wt[:, o, :, 1],
                                               rwb[:, 1:2], wt[:, o, :, 0],
                                               ALU.mult, ALU.add)
                acc2 = wp.tile([128, NCH, COUT], FP16, tag="macc2",
                               name=f"acc2_{b}_{o}")
                nc.vector.scalar_tensor_tensor(acc2[:], wt[:, o, :, 2],
                                               rwb[:, 2:3], acc1[:],
                                               ALU.mult, ALU.add)
                wtile = wp.tile([128, NCH, COUT], FP16, tag=f"weff_{o}",
                                name=f"weff_{b}_{o}")
                nc.vector.scalar_tensor_tensor(wtile[:], wt[:, o, :, 3],
                                               rwb[:, 3:4], acc2[:],
                                               ALU.mult, ALU.add)
                return wtile

            def conv_rhs(b, c, o, nh):
                kh, kw = divmod(o, 3)
                return xp[:, b, c].rearrange("p (h w) -> p h w", h=HP)[
                    :, kh + 16 * nh:kh + 16 * nh + 16, kw:kw + 32]

            def drain(b, m, nh, psum):
                osb = ob.tile([128, 512], FP16, tag=f"osb_{m}_{nh}",
                              name=f"osb_{b}_{m}_{nh}")
                nc.scalar.copy(osb[:], psum[:])
                nc.sync.dma_start(out_d[b, m][:, nh * 512:(nh + 1) * 512],
                                  osb[:])

            # ================= schedule =================
            # warm the PE while x0 lands and pooling runs
            dummies(16)
            emit_pooled(0, "dve")
            dummies(4)
            emit_router(0)
            dummies(10)

            # ---- sample 0: offset-outer, consume weight chunks as they land
            psums0 = {}
            for m in range(MCH):
                for nh in range(2):
                    psums0[(m, nh)] = ps.tile([128, 512], F32, tag="ps8",
                                              name=f"cps_0_{m}_{nh}")
            for o in range(9):
                wtile = mix_weff(0, o)
                for c in range(NCH):
                    for m in range(MCH):
                        lhsT = wtile[:, c, m * 128:(m + 1) * 128]
                        for nh in range(2):
                            nc.tensor.matmul(
                                psums0[(m, nh)], lhsT, conv_rhs(0, c, o, nh),
                                start=(o == 0 and c == 0),
                                stop=(o == 8 and c == NCH - 1))
            # pooling for later samples on ACT/Pool, routers chained on DVE
            emit_pooled(1, "act")
            emit_pooled(2, "act")
            emit_pooled(3, "act")
            emit_router(1)
            for m in range(MCH):
                for nh in range(2):
                    drain(0, m, nh, psums0[(m, nh)])

            mixes = {}
            for o in range(9):
                mixes[(1, o)] = mix_weff(1, o)
            emit_router(2)
            for o in range(9):
                mixes[(2, o)] = mix_weff(2, o)
            emit_router(3)
            for o in range(9):
                mixes[(3, o)] = mix_weff(3, o)

            # ---- samples 1-3: m-sequential groups so drains overlap matmuls
            for b in (1, 2, 3):
                for m in range(MCH):
                    for nh in range(2):
                        psum = ps.tile([128, 512], F32, tag="ps8",
                                       name=f"cps_{b}_{m}_{nh}")
                        first = True
                        for o in range(9):
                            for c in range(NCH):
                                nc.tensor.matmul(
                                    psum[:], mixes[(b, o)][:, c, m * 128:(m + 1) * 128],
                                    conv_rhs(b, c, o, nh), start=first,
                                    stop=(o == 8 and c == NCH - 1))
                                first = False
                        drain(b, m, nh, psum)
    nc.compile()
    return nc


_PROGRAM = None


def _get_program():
    global _PROGRAM
    if _PROGRAM is None:
        _PROGRAM = build_program()
    return _PROGRAM


def _prep_shared(weight, Wq, bq, Wk, bk, Wv, bv, Wm1, bm1, Wm2, bm2, Wc, bc):
    # wt[p, o, c, e, cout] = weight[e, cout, c*128+p, kh, kw]
    w = weight.transpose(2, 3, 4, 0, 1)                   # (CIN,3,3,E,COUT)
    w = w.reshape(NCH, 128, 3, 3, E, COUT).transpose(1, 2, 3, 0, 4, 5)
    wt = np.ascontiguousarray(w.reshape(128, 9, NCH, E, COUT), dtype=np.float32)
    # delta form: slot e>0 := W_e - W_0 (softmax weights sum to 1)
    wt[:, :, :, 1:] -= wt[:, :, :, 0:1]

    rp = np.zeros((128, NPARAM), dtype=np.float32)
    WqT = Wq.T.reshape(NCH, 128, HID)                     # [c,p,j]
    WkT = (Wk / float(PIX)).T.reshape(NCH, 128, HID)
    WvT = (Wv / float(PIX)).T.reshape(NCH, 128, HID)
    for c in range(NCH):
        rp[:, c * HID:(c + 1) * HID] = WqT[c]
        rp[:, 128 + c * HID:128 + (c + 1) * HID] = WkT[c]
        rp[:, 256 + c * HID:256 + (c + 1) * HID] = WvT[c]
    rp[0:HID, 384:448] = Wm1.T
    rp[0:HID, 448:512] = Wm2.T
    rp[0:HID, 512:516] = Wc.T
    rp[HID, 512:516] = bc
    rp[0:HID, 516] = bq
    rp[0:HID, 517] = bk
    rp[0:HID, 518] = bv
    rp[0:HID, 519] = bm1
    rp[0:HID, 520] = bm2
    return wt, rp


def kernel(x, time_emb, weight, Wq, bq, Wk, bk, Wv, bv, Wm1, bm1, Wm2, bm2,
           Wc, bc):
    x = np.asarray(x, dtype=np.float32)
    time_emb = np.asarray(time_emb, dtype=np.float32)
    wt, rp = _prep_shared(np.asarray(weight, np.float32),
                          np.asarray(Wq, np.float32), np.asarray(bq, np.float32),
                          np.asarray(Wk, np.float32), np.asarray(bk, np.float32),
                          np.asarray(Wv, np.float32), np.asarray(bv, np.float32),
                          np.asarray(Wm1, np.float32), np.asarray(bm1, np.float32),
                          np.asarray(Wm2, np.float32), np.asarray(bm2, np.float32),
                          np.asarray(Wc, np.float32), np.asarray(bc, np.float32))

    wt_dev = wt.astype(np.float16)
    in_maps = []
    for i in range(NCORES):
        xl = x[i * BLOC:(i + 1) * BLOC]                   # (4,256,32,32)
        xr = xl.reshape(BLOC, NCH, 128, H, W).transpose(0, 2, 1, 3, 4).astype(np.float16)
        xpad = np.zeros((BLOC, 128, NCH, HP, WP), dtype=np.float16)
        xpad[:, :, :, 1:H + 1, 1:W + 1] = xr
        xpad = np.ascontiguousarray(xpad.reshape(BLOC, 128, NCH, HP * WP))

        tl = time_emb[i * BLOC:(i + 1) * BLOC]            # (4,256)
        te = np.ascontiguousarray(
            tl.T.reshape(NCH, 128, BLOC).transpose(1, 0, 2))

        in_maps.append({"xpad": xpad, "temb": te, "wt": wt_dev, "rparams": rp})

    nc = _get_program()
    res = run_bass_kernel_spmd(nc, in_maps, list(range(NCORES))).results

    y = np.empty((B, COUT, H, W), dtype=np.float32)
    for i in range(NCORES):
        y[i * BLOC:(i + 1) * BLOC] = res[i]["out"].astype(np.float32).reshape(
            BLOC, COUT, H, W)
    return y
